# revision 10
# baseline (speedup 1.0000x reference)
"""Trainium2 Bass kernel for nn_CognitiveModule (gnn_message_passing).

Computes, for L=8 layers of a 1536x1536 grid:
  internal = conv2d(prev_spikes, local_kernel, SAME)      # 11x11 distance kernel
  axonal   = segment_sum(prev_spikes[conn_src] * inter_weights, conn_dst)
  total    = external + internal + axonal
  active   = (refractory == 0)
  v_new    = 0.9 * membrane + active * total
  spikes   = (v_new > 0) * active          (the sigmoid straight-through term
                                            cancels in the forward pass)

Strategy (8 NeuronCores, shard H):
  - Each core gets 192 rows of every layer (plus a 5-row halo of prev_spikes).
  - Conv is computed on the TensorEngine as banded matmuls over the row
    (partition) dimension: for each kernel column kx, a [106,96] band matrix
    contracts 106 input rows into 96 output rows.  The x-taps are reduced from
    11 to 6 matmul passes by exploiting the kernel's x-symmetry: the host pairs
    columns kx=5+d / 5-d and the device pre-adds the shifted spike images
    (S_d = X_{+d} + X_{-d}; spikes are {0,1} so sums are exact in bf16).
  - All matmul data is bf16.  The kernel/weight values are split hi/lo
    (w = bf16(w) + bf16(w - bf16(w))) so products against {0,1,2} spikes are
    exact to ~2^-18 relative - fp32-class accuracy at bf16 matmul speed.
  - external + 0.9*membrane and the refractory gate are folded on the host into
    a single fp32 threshold plane  thr = BIG*(refr != 0) - (ext + 0.9*mem),
    so the device finalize is ONE VectorEngine op per tile:
        out = (psum > thr)    as {1.0, 0.0}.
  - Inter-layer (axonal) products are computed on the VectorEngine from the
    already-resident spike tiles and accumulated into the same PSUM tile via
    bf16 identity matmuls.  Connections with src>dst get a dedicated small
    spike load at the start of each row-block so every dst sees its inputs.
"""

import sys

for _p in ("/opt/trn_rl_repo", "/root/.axon_site/_ro/trn_rl_repo"):
    if _p not in sys.path:
        sys.path.append(_p)

import numpy as np
import ml_dtypes

import concourse.bass as bass
import concourse.mybir as mybir
import concourse.tile as tile
from concourse import bacc
from concourse.bass_utils import run_bass_kernel_spmd

BF16 = mybir.dt.bfloat16
F32 = mybir.dt.float32
BIG = np.float32(1.0e5)
DECAY = np.float32(0.9)

L = 8
NCORES = 8
TH = 96          # output rows per conv tile
HALO = 5
KS = 11          # kernel size
KR = TH + 2 * HALO  # 106 input rows per conv tile
NFREE = 512      # psum free-dim tile

# engine used for each symmetric-pair pre-add (d=1..5); tuning knob.
# d=1,3,5 read the even-aligned X copy; d=2,4 read the odd-aligned copy, so
# every DVE tensor_tensor is 4B-aligned and hits the 2x perf mode.
PREADD_ENGINE = {1: "vector", 2: "gpsimd", 3: "vector", 4: "gpsimd", 5: "vector"}


def _split_bf16(x):
    hi = x.astype(ml_dtypes.bfloat16)
    lo = (x - hi.astype(np.float32)).astype(ml_dtypes.bfloat16)
    return hi, lo


def _group_kernel_columns(kern):
    """Group the 11 kernel columns by x-symmetry.

    Returns a list of (d, col) where d is the x-offset (rhs = S_d for paired
    columns, a direct shifted window for singletons encoded as d=None,dx)."""
    groups = []
    used = [False] * KS
    for d in range(0, HALO + 1):
        a, b = HALO + d, HALO - d
        if d == 0:
            groups.append(("pair", 0, kern[:, HALO].copy()))
            used[HALO] = True
        elif np.array_equal(kern[:, a], kern[:, b]):
            groups.append(("pair", d, kern[:, a].copy()))
            used[a] = used[b] = True
    for kx in range(KS):
        if not used[kx]:
            groups.append(("single", kx - HALO, kern[:, kx].copy()))
    return groups


def _band_matrix(col):
    """[KR, TH] band matrix for the row-direction conv.

    X-tile partition layout: p in [0,101) holds spike row r0+p (out rows and
    bottom halo); p in [101,106) holds the TOP halo rows r0-5..r0-1.  This
    keeps the out-row-aligned rows at partition offset 0 (engine APs cannot
    start at partition 5)."""
    B = np.zeros((KR, TH), np.float32)
    for m in range(TH):
        for ky in range(KS):
            rel = m + ky - HALO          # spike row offset from r0
            p = rel if rel >= 0 else rel + KR
            B[p, m] = col[ky]
    return B


def _build_program(conns, R, W, groups_meta):
    """Build the SPMD Bass program (identical on all cores).

    conns: list of (src, dst) ints.
    R: rows per core (multiple of TH).  W: width (multiple of NFREE).
    groups_meta: list of ("pair", d) / ("single", dx) - band order."""
    nc = bacc.Bacc(None, target_bir_lowering=False, debug=False)
    NT = W // NFREE
    HT = R // TH
    NG = len(groups_meta)  # band groups; 2*NG band matmul passes (hi+lo)

    spk_d = nc.dram_tensor("spk", [L, R + 2 * HALO, W + 2 * HALO], BF16,
                           kind="ExternalInput")
    thr_d = nc.dram_tensor("thr", [L, R, W], F32, kind="ExternalInput")
    # w hi/lo packed side by side: [C, R, 0:W]=hi, [C, R, W:2W]=lo
    wpk_d = nc.dram_tensor("wpk", [len(conns), R, 2 * W], BF16,
                           kind="ExternalInput")
    bands_d = nc.dram_tensor("bands", [KR, 2 * NG * TH], BF16, kind="ExternalInput")
    iden_d = nc.dram_tensor("iden", [TH, TH], BF16, kind="ExternalInput")
    out_d = nc.dram_tensor("out", [L, R, W], BF16, kind="ExternalOutput")

    # connection bookkeeping
    pre_conns = [i for i, (s, d) in enumerate(conns) if s >= d]   # need pre-load
    inline_conns = [i for i, (s, d) in enumerate(conns) if s < d]
    by_src = {}
    for i in inline_conns:
        by_src.setdefault(conns[i][0], []).append(i)
    by_dst = {}
    for i in range(len(conns)):
        by_dst.setdefault(conns[i][1], []).append(i)

    with tile.TileContext(nc) as tc:
        with (
            tc.tile_pool(name="const", bufs=1) as constp,
            tc.tile_pool(name="xp", bufs=2) as xp,
            tc.tile_pool(name="sp", bufs=2) as sp,
            tc.tile_pool(name="thrp", bufs=2) as thrp,
            tc.tile_pool(name="wp", bufs=6) as wp,
            tc.tile_pool(name="cp", bufs=12) as cp,
            tc.tile_pool(name="op", bufs=2) as op,
            tc.tile_pool(name="prep", bufs=2) as prep,
            tc.tile_pool(name="ps", bufs=2, space="PSUM") as psp,
        ):
            bands_sb = constp.tile([KR, 2 * NG * TH], BF16)
            nc.sync.dma_start(out=bands_sb[:], in_=bands_d[:])
            iden_sb = constp.tile([TH, TH], BF16)
            nc.sync.dma_start(out=iden_sb[:], in_=iden_d[:])

            for h in range(HT):
                r0 = h * TH
                contrib = {}  # conn idx -> (hi_tile, lo_tile)

                # --- connections whose src comes later in the layer loop:
                # load just the needed spike rows now.
                for ci in pre_conns:
                    s = conns[ci][0]
                    spre = prep.tile([TH, W], BF16, tag="spre")
                    nc.scalar.dma_start(
                        out=spre[:],
                        in_=spk_d[s, r0 + HALO:r0 + HALO + TH, HALO:HALO + W])
                    wt = wp.tile([TH, 2 * W], BF16, tag="w")
                    nc.sync.dma_start(out=wt[:], in_=wpk_d[ci, r0:r0 + TH, :])
                    chi = cp.tile([TH, W], BF16, tag="c")
                    clo = cp.tile([TH, W], BF16, tag="c")
                    nc.vector.tensor_tensor(out=chi[:], in0=spre[:],
                                            in1=wt[:, 0:W],
                                            op=mybir.AluOpType.mult)
                    nc.vector.tensor_tensor(out=clo[:], in0=spre[:],
                                            in1=wt[:, W:2 * W],
                                            op=mybir.AluOpType.mult)
                    contrib[ci] = (chi, clo)

                for l in range(L):
                    # X partitions 0..100 = spike rows r0..r0+100 (dram rows
                    # r0+5..), partitions 101..105 = top halo (dram r0..r0+4).
                    # Xo is the same image shifted one column left (odd
                    # alignment) so even-d shifts stay 4B-aligned on DVE.
                    X = xp.tile([KR, W + 2 * HALO], BF16, tag="X")
                    nc.scalar.dma_start(
                        out=X[0:KR - HALO, :],
                        in_=spk_d[l, r0 + HALO:r0 + KR, :])
                    nc.scalar.dma_start(
                        out=X[KR - HALO:KR, :],
                        in_=spk_d[l, r0:r0 + HALO, :])
                    Xo = xp.tile([KR, W + 2 * HALO], BF16, tag="Xo")
                    nc.scalar.dma_start(
                        out=Xo[0:KR - HALO, 0:2 * HALO + W - 1],
                        in_=spk_d[l, r0 + HALO:r0 + KR, 1:2 * HALO + W])
                    nc.scalar.dma_start(
                        out=Xo[KR - HALO:KR, 0:2 * HALO + W - 1],
                        in_=spk_d[l, r0:r0 + HALO, 1:2 * HALO + W])

                    # symmetric pre-adds S_d = X_{-d} + X_{+d}
                    svec = {}
                    for gi, (kind, d) in enumerate(groups_meta):
                        if kind == "pair" and d > 0:
                            S = sp.tile([KR, W], BF16, tag=f"S{d}")
                            eng = (nc.gpsimd if PREADD_ENGINE.get(d) == "gpsimd"
                                   else nc.vector)
                            if d % 2 == 0:  # aligned via odd copy
                                eng.tensor_tensor(
                                    out=S[:],
                                    in0=Xo[:, HALO - d - 1:HALO - d - 1 + W],
                                    in1=Xo[:, HALO + d - 1:HALO + d - 1 + W],
                                    op=mybir.AluOpType.add)
                            else:
                                eng.tensor_tensor(
                                    out=S[:], in0=X[:, HALO - d:HALO - d + W],
                                    in1=X[:, HALO + d:HALO + d + W],
                                    op=mybir.AluOpType.add)
                            svec[d] = S

                    thr_t = thrp.tile([TH, W], F32, tag="thr")
                    nc.sync.dma_start(out=thr_t[:], in_=thr_d[l, r0:r0 + TH, :])

                    # contrib planes for connections with src == l (dst > l);
                    # Xo[0:TH, 4:4+W] is the 4B-aligned out-row spike view
                    for ci in by_src.get(l, []):
                        wt = wp.tile([TH, 2 * W], BF16, tag="w")
                        nc.sync.dma_start(out=wt[:],
                                          in_=wpk_d[ci, r0:r0 + TH, :])
                        xs = Xo[0:TH, HALO - 1:HALO - 1 + W]
                        chi = cp.tile([TH, W], BF16, tag="c")
                        clo = cp.tile([TH, W], BF16, tag="c")
                        nc.vector.tensor_tensor(out=chi[:], in0=xs,
                                                in1=wt[:, 0:W],
                                                op=mybir.AluOpType.mult)
                        nc.vector.tensor_tensor(out=clo[:], in0=xs,
                                                in1=wt[:, W:2 * W],
                                                op=mybir.AluOpType.mult)
                        contrib[ci] = (chi, clo)

                    out_t = op.tile([TH, W], BF16, tag="out")
                    my_contribs = [contrib[ci] for ci in by_dst.get(l, [])]
                    ps = psp.tile([TH, W], F32)  # 3 PSUM banks

                    for n in range(NT):
                        c0 = n * NFREE
                        n_mm = 2 * NG + 2 * len(my_contribs)
                        mm = 0
                        # group 0 first (depends only on X), then paired
                        # groups hi+lo, then contribs - keeps PE fed while
                        # pre-adds/muls finish.
                        order = []
                        for gi, (kind, d) in enumerate(groups_meta):
                            order.append((0, gi, kind, d))
                            order.append((1, gi, kind, d))
                        for part, gi, kind, d in order:
                            lhsT = bands_sb[:, (part * NG + gi) * TH:
                                            (part * NG + gi + 1) * TH]
                            if kind == "pair" and d > 0:
                                rhs = svec[d][:, c0:c0 + NFREE]
                            else:
                                dx = 0 if kind == "pair" else d
                                rhs = X[:, HALO + dx + c0:
                                        HALO + dx + c0 + NFREE]
                            nc.tensor.matmul(ps[:, c0:c0 + NFREE], lhsT, rhs,
                                             start=(mm == 0),
                                             stop=(mm == n_mm - 1))
                            mm += 1
                        for chi, clo in my_contribs:
                            for ct in (chi, clo):
                                nc.tensor.matmul(ps[:, c0:c0 + NFREE],
                                                 iden_sb[:],
                                                 ct[:, c0:c0 + NFREE],
                                                 start=(mm == 0),
                                                 stop=(mm == n_mm - 1))
                                mm += 1
                    nc.vector.tensor_tensor(
                        out=out_t[:], in0=ps[:], in1=thr_t[:],
                        op=mybir.AluOpType.is_gt)

                    nc.sync.dma_start(out=out_d[l, r0:r0 + TH, :], in_=out_t[:])

    nc.compile()
    return nc


_PROGRAM_CACHE = {}


def _get_program(conns, R, W, groups_meta):
    key = (tuple(conns), R, W, tuple(k for k, _d in [(g[0], g[1]) for g in
                                                     groups_meta]),
           tuple(g[1] for g in groups_meta))
    if key not in _PROGRAM_CACHE:
        _PROGRAM_CACHE[key] = _build_program(conns, R, W, groups_meta)
    return _PROGRAM_CACHE[key]


def _prepare_inputs(external, prev_spikes, membrane, inter_weights,
                    local_kernel, refractory, conn_src, conn_dst):
    Lx, H, W = external.shape
    R = H // NCORES
    conns = [(int(s), int(d)) for s, d in zip(conn_src, conn_dst)]

    groups = _group_kernel_columns(np.asarray(local_kernel, np.float32))
    groups_meta = [(k, d) for k, d, _c in groups]

    # band matrices, hi parts then lo parts, [KR, 2*NG*TH] bf16
    NG = len(groups)
    bands = np.zeros((KR, 2 * NG * TH), ml_dtypes.bfloat16)
    for gi, (_k, _d, col) in enumerate(groups):
        B = _band_matrix(col)
        hi, lo = _split_bf16(B)
        bands[:, gi * TH:(gi + 1) * TH] = hi
        bands[:, (NG + gi) * TH:(NG + gi + 1) * TH] = lo
    iden = np.eye(TH, dtype=ml_dtypes.bfloat16)

    # fp32 threshold plane: out fires iff psum > thr
    ext = np.asarray(external, np.float32)
    mem = np.asarray(membrane, np.float32)
    refr = np.asarray(refractory)
    thr = (BIG * (refr != 0).astype(np.float32)
           - (ext + DECAY * mem)).astype(np.float32)

    # padded bf16 spikes (exact: values {0,1})
    spk = np.zeros((Lx, H + 2 * HALO, W + 2 * HALO), ml_dtypes.bfloat16)
    spk[:, HALO:H + HALO, HALO:W + HALO] = np.asarray(prev_spikes, np.float32)

    w_hi, w_lo = _split_bf16(np.asarray(inter_weights, np.float32))
    wpk = np.concatenate([w_hi, w_lo], axis=2)  # [C, H, 2W]

    in_maps = []
    for c in range(NCORES):
        g0 = c * R
        in_maps.append({
            "spk": np.ascontiguousarray(spk[:, g0:g0 + R + 2 * HALO, :]),
            "thr": np.ascontiguousarray(thr[:, g0:g0 + R, :]),
            "wpk": np.ascontiguousarray(wpk[:, g0:g0 + R, :]),
            "bands": bands,
            "iden": iden,
        })
    return conns, R, W, groups_meta, in_maps


def _ensure_ntff_hook():
    """Inject the missing antenv.axon_hooks module + ctypes NTFF hook so
    trace=True works in this image (profiling only; best-effort)."""
    import types
    try:
        import antenv.axon_hooks  # noqa: F401
        return
    except ImportError:
        pass
    try:
        import antenv
        mod = types.ModuleType("antenv.axon_hooks")
        _h = [None]
        mod.set_axon_ntff_profile_hook = lambda h: _h.__setitem__(0, h)
        mod.get_axon_ntff_profile_hook = lambda: _h[0]
        sys.modules["antenv.axon_hooks"] = mod
        antenv.axon_hooks = mod
        from trn_agent_boot.trn_boot import _ntff_profile_via_ctypes
        hook = _ntff_profile_via_ctypes("/opt/axon/libaxon_pjrt.so")
        if hook is not None:
            _h[0] = hook
    except Exception:
        pass


def kernel(external, prev_spikes, membrane, inter_weights, local_kernel,
           refractory, conn_src, conn_dst, _trace=False):
    if _trace:
        _ensure_ntff_hook()
    conns, R, W, groups_meta, in_maps = _prepare_inputs(
        external, prev_spikes, membrane, inter_weights, local_kernel,
        refractory, conn_src, conn_dst)
    nc = _get_program(conns, R, W, groups_meta)
    res = run_bass_kernel_spmd(nc, in_maps, core_ids=list(range(NCORES)),
                               trace=_trace)
    out = np.concatenate([r["out"].astype(np.float32) for r in res.results],
                         axis=1)
    if _trace:
        kernel._last_results = res
    return out


# revision 13
# speedup vs baseline: 1.5006x; 1.5006x over previous
"""Trainium2 Bass kernel for nn_CognitiveModule (gnn_message_passing).

Computes, for L=8 layers of a 1536x1536 grid:
  internal = conv2d(prev_spikes, local_kernel, SAME)      # 11x11 distance kernel
  axonal   = segment_sum(prev_spikes[conn_src] * inter_weights, conn_dst)
  total    = external + internal + axonal
  active   = (refractory == 0)
  v_new    = 0.9 * membrane + active * total
  spikes   = (v_new > 0) * active          (the sigmoid straight-through term
                                            cancels in the forward pass)

Strategy (8 NeuronCores, shard H):
  - Each core gets 192 rows of every layer (plus a 5-row halo of prev_spikes).
  - Conv is computed on the TensorEngine as banded matmuls over the row
    (partition) dimension: for each kernel column kx, a [106,96] band matrix
    contracts 106 input rows into 96 output rows.  The x-taps are reduced from
    11 to 6 matmul passes by exploiting the kernel's x-symmetry: the host pairs
    columns kx=5+d / 5-d and the device pre-adds the shifted spike images
    (S_d = X_{+d} + X_{-d}; spikes are {0,1} so sums are exact in bf16).
  - All matmul data is bf16.  The kernel/weight values are split hi/lo
    (w = bf16(w) + bf16(w - bf16(w))) so products against {0,1,2} spikes are
    exact to ~2^-18 relative - fp32-class accuracy at bf16 matmul speed.
  - external + 0.9*membrane and the refractory gate are folded on the host into
    a single fp32 threshold plane  thr = BIG*(refr != 0) - (ext + 0.9*mem),
    so the device finalize is ONE VectorEngine op per tile:
        out = (psum > thr)    as {1.0, 0.0}.
  - Inter-layer (axonal) products are computed on the VectorEngine from the
    already-resident spike tiles and accumulated into the same PSUM tile via
    bf16 identity matmuls.  Connections with src>dst get a dedicated small
    spike load at the start of each row-block so every dst sees its inputs.
"""

import sys

for _p in ("/opt/trn_rl_repo", "/root/.axon_site/_ro/trn_rl_repo"):
    if _p not in sys.path:
        sys.path.append(_p)

import numpy as np
import ml_dtypes

import concourse.bass as bass
import concourse.mybir as mybir
import concourse.tile as tile
from concourse import bacc
from concourse.bass_utils import run_bass_kernel_spmd

BF16 = mybir.dt.bfloat16
F32 = mybir.dt.float32
BIG = np.float32(1.0e5)
DECAY = np.float32(0.9)

L = 8
NCORES = 8
TH = 96          # output rows per conv tile
HALO = 5
KS = 11          # kernel size
KR = TH + 2 * HALO  # 106 input rows per conv tile
NFREE = 512      # psum free-dim tile

# engine used for each symmetric-pair pre-add (d=1..5); tuning knob.
# d=1,3,5 read the even-aligned X copy; d=2,4 read the odd-aligned copy, so
# every DVE tensor_tensor is 4B-aligned and hits the 2x perf mode.
PREADD_ENGINE = {1: "vector", 2: "gpsimd", 3: "vector", 4: "gpsimd", 5: "vector"}


def _split_bf16(x):
    hi = x.astype(ml_dtypes.bfloat16)
    lo = (x - hi.astype(np.float32)).astype(ml_dtypes.bfloat16)
    return hi, lo


def _group_kernel_columns(kern):
    """Group the 11 kernel columns by x-symmetry.

    Returns a list of (d, col) where d is the x-offset (rhs = S_d for paired
    columns, a direct shifted window for singletons encoded as d=None,dx)."""
    groups = []
    used = [False] * KS
    for d in range(0, HALO + 1):
        a, b = HALO + d, HALO - d
        if d == 0:
            groups.append(("pair", 0, kern[:, HALO].copy()))
            used[HALO] = True
        elif np.array_equal(kern[:, a], kern[:, b]):
            groups.append(("pair", d, kern[:, a].copy()))
            used[a] = used[b] = True
    for kx in range(KS):
        if not used[kx]:
            groups.append(("single", kx - HALO, kern[:, kx].copy()))
    return groups


def _band_matrix(col):
    """[KR, TH] band matrix for the row-direction conv.

    X-tile partition layout: p in [0,101) holds spike row r0+p (out rows and
    bottom halo); p in [101,106) holds the TOP halo rows r0-5..r0-1.  This
    keeps the out-row-aligned rows at partition offset 0 (engine APs cannot
    start at partition 5)."""
    B = np.zeros((KR, TH), np.float32)
    for m in range(TH):
        for ky in range(KS):
            rel = m + ky - HALO          # spike row offset from r0
            p = rel if rel >= 0 else rel + KR
            B[p, m] = col[ky]
    return B


def _build_program(conns, R, W, groups_meta):
    """Build the SPMD Bass program (identical on all cores).

    conns: list of (src, dst) ints.
    R: rows per core (multiple of TH).  W: width (multiple of NFREE).
    groups_meta: list of ("pair", d) / ("single", dx) - band order."""
    nc = bacc.Bacc(None, target_bir_lowering=False, debug=False)
    NT = W // NFREE
    HT = R // TH
    NG = len(groups_meta)  # band groups; 2*NG band matmul passes (hi+lo)

    spk_d = nc.dram_tensor("spk", [L, R + 2 * HALO, W + 2 * HALO], BF16,
                           kind="ExternalInput")
    thr_d = nc.dram_tensor("thr", [L, R, W], F32, kind="ExternalInput")
    # w hi/lo packed side by side: [C, R, 0:W]=hi, [C, R, W:2W]=lo
    wpk_d = nc.dram_tensor("wpk", [len(conns), R, 2 * W], BF16,
                           kind="ExternalInput")
    bands_d = nc.dram_tensor("bands", [KR, 2 * NG * TH], BF16, kind="ExternalInput")
    iden_d = nc.dram_tensor("iden", [TH, TH], BF16, kind="ExternalInput")
    out_d = nc.dram_tensor("out", [L, R, W], BF16, kind="ExternalOutput")

    # connection bookkeeping
    pre_conns = [i for i, (s, d) in enumerate(conns) if s >= d]   # need pre-load
    inline_conns = [i for i, (s, d) in enumerate(conns) if s < d]
    by_src = {}
    for i in inline_conns:
        by_src.setdefault(conns[i][0], []).append(i)
    by_dst = {}
    for i in range(len(conns)):
        by_dst.setdefault(conns[i][1], []).append(i)

    with tile.TileContext(nc) as tc:
        with (
            tc.tile_pool(name="const", bufs=1) as constp,
            tc.tile_pool(name="xp", bufs=3) as xp,
            tc.tile_pool(name="sp", bufs=2) as sp,
            tc.tile_pool(name="thrp", bufs=3) as thrp,
            tc.tile_pool(name="wp", bufs=6) as wp,
            tc.tile_pool(name="cp", bufs=12) as cp,
            tc.tile_pool(name="op", bufs=3) as op,
            tc.tile_pool(name="prep", bufs=2) as prep,
            tc.tile_pool(name="ps", bufs=2, space="PSUM") as psp,
        ):
            bands_sb = constp.tile([KR, 2 * NG * TH], BF16)
            nc.sync.dma_start(out=bands_sb[:], in_=bands_d[:])
            iden_sb = constp.tile([TH, TH], BF16)
            nc.sync.dma_start(out=iden_sb[:], in_=iden_d[:])

            for h in range(HT):
                r0 = h * TH
                contrib = {}  # conn idx -> (hi_tile, lo_tile)

                # --- connections whose src comes later in the layer loop:
                # load just the needed spike rows now.
                for ci in pre_conns:
                    s = conns[ci][0]
                    spre = prep.tile([TH, W], BF16, tag="spre")
                    nc.scalar.dma_start(
                        out=spre[:],
                        in_=spk_d[s, r0 + HALO:r0 + HALO + TH, HALO:HALO + W])
                    wt = wp.tile([TH, 2 * W], BF16, tag="w")
                    nc.sync.dma_start(out=wt[:], in_=wpk_d[ci, r0:r0 + TH, :])
                    chi = cp.tile([TH, W], BF16, tag="c")
                    clo = cp.tile([TH, W], BF16, tag="c")
                    nc.vector.tensor_tensor(out=chi[:], in0=spre[:],
                                            in1=wt[:, 0:W],
                                            op=mybir.AluOpType.mult)
                    nc.vector.tensor_tensor(out=clo[:], in0=spre[:],
                                            in1=wt[:, W:2 * W],
                                            op=mybir.AluOpType.mult)
                    contrib[ci] = (chi, clo)

                for l in range(L):
                    # X partitions 0..100 = spike rows r0..r0+100 (dram rows
                    # r0+5..), partitions 101..105 = top halo (dram r0..r0+4).
                    # Xo is the same image shifted one column left (odd
                    # alignment) so even-d shifts stay 4B-aligned on DVE.
                    X = xp.tile([KR, W + 2 * HALO], BF16, tag="X")
                    nc.scalar.dma_start(
                        out=X[0:KR - HALO, :],
                        in_=spk_d[l, r0 + HALO:r0 + KR, :])
                    nc.scalar.dma_start(
                        out=X[KR - HALO:KR, :],
                        in_=spk_d[l, r0:r0 + HALO, :])
                    # odd-aligned out-row spike copy (contrib muls only)
                    Xo = xp.tile([TH, 2 * HALO + W - 1], BF16, tag="Xo")
                    nc.sync.dma_start(
                        out=Xo[:],
                        in_=spk_d[l, r0 + HALO:r0 + HALO + TH,
                                  1:2 * HALO + W])

                    # symmetric pre-adds S_d = X_{-d} + X_{+d}
                    svec = {}
                    for gi, (kind, d) in enumerate(groups_meta):
                        if kind == "pair" and d > 0:
                            S = sp.tile([KR, W], BF16, tag=f"S{d}")
                            eng = (nc.gpsimd if PREADD_ENGINE.get(d) == "gpsimd"
                                   else nc.vector)
                            eng.tensor_tensor(
                                out=S[:], in0=X[:, HALO - d:HALO - d + W],
                                in1=X[:, HALO + d:HALO + d + W],
                                op=mybir.AluOpType.add)
                            svec[d] = S

                    thr_t = thrp.tile([TH, W], F32, tag="thr")
                    nc.sync.dma_start(out=thr_t[:], in_=thr_d[l, r0:r0 + TH, :])

                    # contrib planes for connections with src == l (dst > l);
                    # Xo[0:TH, 4:4+W] is the 4B-aligned out-row spike view
                    for ci in by_src.get(l, []):
                        wt = wp.tile([TH, 2 * W], BF16, tag="w")
                        nc.sync.dma_start(out=wt[:],
                                          in_=wpk_d[ci, r0:r0 + TH, :])
                        xs = Xo[0:TH, HALO - 1:HALO - 1 + W]
                        chi = cp.tile([TH, W], BF16, tag="c")
                        clo = cp.tile([TH, W], BF16, tag="c")
                        nc.vector.tensor_tensor(out=chi[:], in0=xs,
                                                in1=wt[:, 0:W],
                                                op=mybir.AluOpType.mult)
                        nc.vector.tensor_tensor(out=clo[:], in0=xs,
                                                in1=wt[:, W:2 * W],
                                                op=mybir.AluOpType.mult)
                        contrib[ci] = (chi, clo)

                    out_t = op.tile([TH, W], BF16, tag="out")
                    my_contribs = [contrib[ci] for ci in by_dst.get(l, [])]
                    ps = psp.tile([TH, W], F32)  # 3 PSUM banks

                    for n in range(NT):
                        c0 = n * NFREE
                        n_mm = 2 * NG + 2 * len(my_contribs)
                        mm = 0
                        # group 0 first (depends only on X), then paired
                        # groups hi+lo, then contribs - keeps PE fed while
                        # pre-adds/muls finish.
                        order = []
                        for gi, (kind, d) in enumerate(groups_meta):
                            order.append((0, gi, kind, d))
                            order.append((1, gi, kind, d))
                        for part, gi, kind, d in order:
                            lhsT = bands_sb[:, (part * NG + gi) * TH:
                                            (part * NG + gi + 1) * TH]
                            if kind == "pair" and d > 0:
                                rhs = svec[d][:, c0:c0 + NFREE]
                            else:
                                dx = 0 if kind == "pair" else d
                                rhs = X[:, HALO + dx + c0:
                                        HALO + dx + c0 + NFREE]
                            nc.tensor.matmul(ps[:, c0:c0 + NFREE], lhsT, rhs,
                                             start=(mm == 0),
                                             stop=(mm == n_mm - 1))
                            mm += 1
                        for chi, clo in my_contribs:
                            for ct in (chi, clo):
                                nc.tensor.matmul(ps[:, c0:c0 + NFREE],
                                                 iden_sb[:],
                                                 ct[:, c0:c0 + NFREE],
                                                 start=(mm == 0),
                                                 stop=(mm == n_mm - 1))
                                mm += 1
                    nc.vector.tensor_tensor(
                        out=out_t[:], in0=ps[:], in1=thr_t[:],
                        op=mybir.AluOpType.is_gt)

                    nc.sync.dma_start(out=out_d[l, r0:r0 + TH, :], in_=out_t[:])

    nc.compile()
    return nc


_PROGRAM_CACHE = {}


def _get_program(conns, R, W, groups_meta):
    key = (tuple(conns), R, W, tuple(k for k, _d in [(g[0], g[1]) for g in
                                                     groups_meta]),
           tuple(g[1] for g in groups_meta))
    if key not in _PROGRAM_CACHE:
        _PROGRAM_CACHE[key] = _build_program(conns, R, W, groups_meta)
    return _PROGRAM_CACHE[key]


def _prepare_inputs(external, prev_spikes, membrane, inter_weights,
                    local_kernel, refractory, conn_src, conn_dst):
    Lx, H, W = external.shape
    R = H // NCORES
    conns = [(int(s), int(d)) for s, d in zip(conn_src, conn_dst)]

    groups = _group_kernel_columns(np.asarray(local_kernel, np.float32))
    groups_meta = [(k, d) for k, d, _c in groups]

    # band matrices, hi parts then lo parts, [KR, 2*NG*TH] bf16
    NG = len(groups)
    bands = np.zeros((KR, 2 * NG * TH), ml_dtypes.bfloat16)
    for gi, (_k, _d, col) in enumerate(groups):
        B = _band_matrix(col)
        hi, lo = _split_bf16(B)
        bands[:, gi * TH:(gi + 1) * TH] = hi
        bands[:, (NG + gi) * TH:(NG + gi + 1) * TH] = lo
    iden = np.eye(TH, dtype=ml_dtypes.bfloat16)

    # fp32 threshold plane: out fires iff psum > thr
    ext = np.asarray(external, np.float32)
    mem = np.asarray(membrane, np.float32)
    refr = np.asarray(refractory)
    thr = (BIG * (refr != 0).astype(np.float32)
           - (ext + DECAY * mem)).astype(np.float32)

    # padded bf16 spikes (exact: values {0,1})
    spk = np.zeros((Lx, H + 2 * HALO, W + 2 * HALO), ml_dtypes.bfloat16)
    spk[:, HALO:H + HALO, HALO:W + HALO] = np.asarray(prev_spikes, np.float32)

    w_hi, w_lo = _split_bf16(np.asarray(inter_weights, np.float32))
    wpk = np.concatenate([w_hi, w_lo], axis=2)  # [C, H, 2W]

    in_maps = []
    for c in range(NCORES):
        g0 = c * R
        in_maps.append({
            "spk": np.ascontiguousarray(spk[:, g0:g0 + R + 2 * HALO, :]),
            "thr": np.ascontiguousarray(thr[:, g0:g0 + R, :]),
            "wpk": np.ascontiguousarray(wpk[:, g0:g0 + R, :]),
            "bands": bands,
            "iden": iden,
        })
    return conns, R, W, groups_meta, in_maps


def _ensure_ntff_hook():
    """Inject the missing antenv.axon_hooks module + ctypes NTFF hook so
    trace=True works in this image (profiling only; best-effort)."""
    import types
    try:
        import antenv.axon_hooks  # noqa: F401
        return
    except ImportError:
        pass
    try:
        import antenv
        mod = types.ModuleType("antenv.axon_hooks")
        _h = [None]
        mod.set_axon_ntff_profile_hook = lambda h: _h.__setitem__(0, h)
        mod.get_axon_ntff_profile_hook = lambda: _h[0]
        sys.modules["antenv.axon_hooks"] = mod
        antenv.axon_hooks = mod
        from trn_agent_boot.trn_boot import _ntff_profile_via_ctypes
        hook = _ntff_profile_via_ctypes("/opt/axon/libaxon_pjrt.so")
        if hook is not None:
            _h[0] = hook
    except Exception:
        pass


def kernel(external, prev_spikes, membrane, inter_weights, local_kernel,
           refractory, conn_src, conn_dst, _trace=False):
    if _trace:
        _ensure_ntff_hook()
    conns, R, W, groups_meta, in_maps = _prepare_inputs(
        external, prev_spikes, membrane, inter_weights, local_kernel,
        refractory, conn_src, conn_dst)
    nc = _get_program(conns, R, W, groups_meta)
    res = run_bass_kernel_spmd(nc, in_maps, core_ids=list(range(NCORES)),
                               trace=_trace)
    out = np.concatenate([r["out"].astype(np.float32) for r in res.results],
                         axis=1)
    if _trace:
        kernel._last_results = res
    return out


# revision 16
# speedup vs baseline: 1.5078x; 1.0048x over previous
"""Trainium2 Bass kernel for nn_CognitiveModule (gnn_message_passing).

Computes, for L=8 layers of a 1536x1536 grid:
  internal = conv2d(prev_spikes, local_kernel, SAME)      # 11x11 distance kernel
  axonal   = segment_sum(prev_spikes[conn_src] * inter_weights, conn_dst)
  total    = external + internal + axonal
  active   = (refractory == 0)
  v_new    = 0.9 * membrane + active * total
  spikes   = (v_new > 0) * active          (the sigmoid straight-through term
                                            cancels in the forward pass)

Strategy (8 NeuronCores, shard H):
  - Each core gets 192 rows of every layer (plus a 5-row halo of prev_spikes).
  - Conv is computed on the TensorEngine as banded matmuls over the row
    (partition) dimension: for each kernel column kx, a [106,96] band matrix
    contracts 106 input rows into 96 output rows.  The x-taps are reduced from
    11 to 6 matmul passes by exploiting the kernel's x-symmetry: the host pairs
    columns kx=5+d / 5-d and the device pre-adds the shifted spike images
    (S_d = X_{+d} + X_{-d}; spikes are {0,1} so sums are exact in bf16).
  - All matmul data is bf16.  The kernel/weight values are split hi/lo
    (w = bf16(w) + bf16(w - bf16(w))) so products against {0,1,2} spikes are
    exact to ~2^-18 relative - fp32-class accuracy at bf16 matmul speed.
  - external + 0.9*membrane and the refractory gate are folded on the host into
    a single fp32 threshold plane  thr = BIG*(refr != 0) - (ext + 0.9*mem),
    so the device finalize is ONE VectorEngine op per tile:
        out = (psum > thr)    as {1.0, 0.0}.
  - Inter-layer (axonal) products are computed on the VectorEngine from the
    already-resident spike tiles and accumulated into the same PSUM tile via
    bf16 identity matmuls.  Connections with src>dst get a dedicated small
    spike load at the start of each row-block so every dst sees its inputs.
"""

import sys

for _p in ("/opt/trn_rl_repo", "/root/.axon_site/_ro/trn_rl_repo"):
    if _p not in sys.path:
        sys.path.append(_p)

import numpy as np
import ml_dtypes

import concourse.bass as bass
import concourse.mybir as mybir
import concourse.tile as tile
from concourse import bacc
from concourse.bass_utils import run_bass_kernel_spmd

BF16 = mybir.dt.bfloat16
F32 = mybir.dt.float32
BIG = np.float32(1.0e5)
DECAY = np.float32(0.9)

L = 8
NCORES = 8
TH = 96          # output rows per conv tile
HALO = 5
KS = 11          # kernel size
KR = TH + 2 * HALO  # 106 input rows per conv tile
NFREE = 512      # psum free-dim tile

# engine used for each symmetric-pair pre-add (d=1..5); tuning knob.
# d=1,3,5 read the even-aligned X copy; d=2,4 read the odd-aligned copy, so
# every DVE tensor_tensor is 4B-aligned and hits the 2x perf mode.
PREADD_ENGINE = {1: "vector", 2: "gpsimd", 3: "vector", 4: "gpsimd", 5: "vector"}


def _split_bf16(x):
    hi = x.astype(ml_dtypes.bfloat16)
    lo = (x - hi.astype(np.float32)).astype(ml_dtypes.bfloat16)
    return hi, lo


def _group_kernel_columns(kern):
    """Group the 11 kernel columns by x-symmetry.

    Returns a list of (d, col) where d is the x-offset (rhs = S_d for paired
    columns, a direct shifted window for singletons encoded as d=None,dx)."""
    groups = []
    used = [False] * KS
    for d in range(0, HALO + 1):
        a, b = HALO + d, HALO - d
        if d == 0:
            groups.append(("pair", 0, kern[:, HALO].copy()))
            used[HALO] = True
        elif np.array_equal(kern[:, a], kern[:, b]):
            groups.append(("pair", d, kern[:, a].copy()))
            used[a] = used[b] = True
    for kx in range(KS):
        if not used[kx]:
            groups.append(("single", kx - HALO, kern[:, kx].copy()))
    return groups


def _band_matrix(col):
    """[KR, TH] band matrix for the row-direction conv.

    X-tile partition layout: p in [0,101) holds spike row r0+p (out rows and
    bottom halo); p in [101,106) holds the TOP halo rows r0-5..r0-1.  This
    keeps the out-row-aligned rows at partition offset 0 (engine APs cannot
    start at partition 5)."""
    B = np.zeros((KR, TH), np.float32)
    for m in range(TH):
        for ky in range(KS):
            rel = m + ky - HALO          # spike row offset from r0
            p = rel if rel >= 0 else rel + KR
            B[p, m] = col[ky]
    return B


def _build_program(conns, R, W, groups_meta):
    """Build the SPMD Bass program (identical on all cores).

    conns: list of (src, dst) ints.
    R: rows per core (multiple of TH).  W: width (multiple of NFREE).
    groups_meta: list of ("pair", d) / ("single", dx) - band order."""
    nc = bacc.Bacc(None, target_bir_lowering=False, debug=False)
    NT = W // NFREE
    HT = R // TH
    NG = len(groups_meta)  # band groups; 2*NG band matmul passes (hi+lo)

    spk_d = nc.dram_tensor("spk", [L, R + 2 * HALO, W + 2 * HALO], BF16,
                           kind="ExternalInput")
    thr_d = nc.dram_tensor("thr", [L, R, W], F32, kind="ExternalInput")
    # w hi/lo packed side by side: [C, R, 0:W]=hi, [C, R, W:2W]=lo
    wpk_d = nc.dram_tensor("wpk", [len(conns), R, 2 * W], BF16,
                           kind="ExternalInput")
    bands_d = nc.dram_tensor("bands", [KR, 2 * NG * TH], BF16, kind="ExternalInput")
    iden_d = nc.dram_tensor("iden", [TH, TH], BF16, kind="ExternalInput")
    out_d = nc.dram_tensor("out", [L, R, W], BF16, kind="ExternalOutput")

    # connection bookkeeping
    pre_conns = [i for i, (s, d) in enumerate(conns) if s >= d]   # need pre-load
    inline_conns = [i for i, (s, d) in enumerate(conns) if s < d]
    by_src = {}
    for i in inline_conns:
        by_src.setdefault(conns[i][0], []).append(i)
    by_dst = {}
    for i in range(len(conns)):
        by_dst.setdefault(conns[i][1], []).append(i)

    with tile.TileContext(nc) as tc:
        with (
            tc.tile_pool(name="const", bufs=1) as constp,
            tc.tile_pool(name="xp", bufs=3) as xp,
            tc.tile_pool(name="sp", bufs=2) as sp,
            tc.tile_pool(name="thrp", bufs=3) as thrp,
            tc.tile_pool(name="wp", bufs=6) as wp,
            tc.tile_pool(name="cp", bufs=12) as cp,
            tc.tile_pool(name="op", bufs=3) as op,
            tc.tile_pool(name="prep", bufs=2) as prep,
            tc.tile_pool(name="ps", bufs=2, space="PSUM") as psp,
        ):
            bands_sb = constp.tile([KR, 2 * NG * TH], BF16)
            nc.sync.dma_start(out=bands_sb[:], in_=bands_d[:])
            iden_sb = constp.tile([TH, TH], BF16)
            nc.sync.dma_start(out=iden_sb[:], in_=iden_d[:])

            # deferred finalize: (ps, thr_t, out_t, l, r0) emitted one layer
            # later so the DVE is_gt never blocks the next layer's pre-adds
            # (and the PE never waits on the DVE queue).
            pending = [None]

            def flush_pending():
                if pending[0] is None:
                    return
                ps_p, thr_p, out_p, l_p, r0_p = pending[0]
                nc.vector.tensor_tensor(
                    out=out_p[:], in0=ps_p[:], in1=thr_p[:],
                    op=mybir.AluOpType.is_gt)
                nc.sync.dma_start(out=out_d[l_p, r0_p:r0_p + TH, :],
                                  in_=out_p[:])
                pending[0] = None

            for h in range(HT):
                r0 = h * TH
                contrib = {}  # conn idx -> (hi_tile, lo_tile)

                # --- connections whose src comes later in the layer loop:
                # load just the needed spike rows now.
                for ci in pre_conns:
                    s = conns[ci][0]
                    spre = prep.tile([TH, W], BF16, tag="spre")
                    nc.scalar.dma_start(
                        out=spre[:],
                        in_=spk_d[s, r0 + HALO:r0 + HALO + TH, HALO:HALO + W])
                    wt = wp.tile([TH, 2 * W], BF16, tag="w")
                    nc.sync.dma_start(out=wt[:], in_=wpk_d[ci, r0:r0 + TH, :])
                    chi = cp.tile([TH, W], BF16, tag="c")
                    clo = cp.tile([TH, W], BF16, tag="c")
                    nc.vector.tensor_tensor(out=chi[:], in0=spre[:],
                                            in1=wt[:, 0:W],
                                            op=mybir.AluOpType.mult)
                    nc.vector.tensor_tensor(out=clo[:], in0=spre[:],
                                            in1=wt[:, W:2 * W],
                                            op=mybir.AluOpType.mult)
                    contrib[ci] = (chi, clo)

                for l in range(L):
                    # X partitions 0..100 = spike rows r0..r0+100 (dram rows
                    # r0+5..), partitions 101..105 = top halo (dram r0..r0+4).
                    # Xo is the same image shifted one column left (odd
                    # alignment) so even-d shifts stay 4B-aligned on DVE.
                    X = xp.tile([KR, W + 2 * HALO], BF16, tag="X")
                    nc.scalar.dma_start(
                        out=X[0:KR - HALO, :],
                        in_=spk_d[l, r0 + HALO:r0 + KR, :])
                    nc.scalar.dma_start(
                        out=X[KR - HALO:KR, :],
                        in_=spk_d[l, r0:r0 + HALO, :])
                    # odd-aligned out-row spike copy (contrib muls only)
                    Xo = xp.tile([TH, 2 * HALO + W - 1], BF16, tag="Xo")
                    nc.sync.dma_start(
                        out=Xo[:],
                        in_=spk_d[l, r0 + HALO:r0 + HALO + TH,
                                  1:2 * HALO + W])

                    # symmetric pre-adds S_d = X_{-d} + X_{+d}
                    svec = {}
                    for gi, (kind, d) in enumerate(groups_meta):
                        if kind == "pair" and d > 0:
                            S = sp.tile([KR, W], BF16, tag=f"S{d}")
                            eng = (nc.gpsimd if PREADD_ENGINE.get(d) == "gpsimd"
                                   else nc.vector)
                            eng.tensor_tensor(
                                out=S[:], in0=X[:, HALO - d:HALO - d + W],
                                in1=X[:, HALO + d:HALO + d + W],
                                op=mybir.AluOpType.add)
                            svec[d] = S

                    thr_t = thrp.tile([TH, W], F32, tag="thr")
                    nc.sync.dma_start(out=thr_t[:], in_=thr_d[l, r0:r0 + TH, :])

                    # contrib planes for connections with src == l (dst > l);
                    # Xo[0:TH, 4:4+W] is the 4B-aligned out-row spike view
                    for ci in by_src.get(l, []):
                        wt = wp.tile([TH, 2 * W], BF16, tag="w")
                        nc.sync.dma_start(out=wt[:],
                                          in_=wpk_d[ci, r0:r0 + TH, :])
                        xs = Xo[0:TH, HALO - 1:HALO - 1 + W]
                        chi = cp.tile([TH, W], BF16, tag="c")
                        clo = cp.tile([TH, W], BF16, tag="c")
                        nc.vector.tensor_tensor(out=chi[:], in0=xs,
                                                in1=wt[:, 0:W],
                                                op=mybir.AluOpType.mult)
                        nc.vector.tensor_tensor(out=clo[:], in0=xs,
                                                in1=wt[:, W:2 * W],
                                                op=mybir.AluOpType.mult)
                        contrib[ci] = (chi, clo)

                    flush_pending()

                    out_t = op.tile([TH, W], BF16, tag="out")
                    my_contribs = [contrib[ci] for ci in by_dst.get(l, [])]
                    ps = psp.tile([TH, W], F32)  # 3 PSUM banks

                    for n in range(NT):
                        c0 = n * NFREE
                        n_mm = 2 * NG + 2 * len(my_contribs)
                        mm = 0
                        # group 0 first (depends only on X), then paired
                        # groups hi+lo, then contribs - keeps PE fed while
                        # pre-adds/muls finish.
                        order = []
                        for gi, (kind, d) in enumerate(groups_meta):
                            order.append((0, gi, kind, d))
                            order.append((1, gi, kind, d))
                        for part, gi, kind, d in order:
                            lhsT = bands_sb[:, (part * NG + gi) * TH:
                                            (part * NG + gi + 1) * TH]
                            if kind == "pair" and d > 0:
                                rhs = svec[d][:, c0:c0 + NFREE]
                            else:
                                dx = 0 if kind == "pair" else d
                                rhs = X[:, HALO + dx + c0:
                                        HALO + dx + c0 + NFREE]
                            nc.tensor.matmul(ps[:, c0:c0 + NFREE], lhsT, rhs,
                                             start=(mm == 0),
                                             stop=(mm == n_mm - 1))
                            mm += 1
                        for chi, clo in my_contribs:
                            for ct in (chi, clo):
                                nc.tensor.matmul(ps[:, c0:c0 + NFREE],
                                                 iden_sb[:],
                                                 ct[:, c0:c0 + NFREE],
                                                 start=(mm == 0),
                                                 stop=(mm == n_mm - 1))
                                mm += 1
                    pending[0] = (ps, thr_t, out_t, l, r0)
            flush_pending()

    nc.compile()
    return nc


_PROGRAM_CACHE = {}


def _get_program(conns, R, W, groups_meta):
    key = (tuple(conns), R, W, tuple(k for k, _d in [(g[0], g[1]) for g in
                                                     groups_meta]),
           tuple(g[1] for g in groups_meta))
    if key not in _PROGRAM_CACHE:
        _PROGRAM_CACHE[key] = _build_program(conns, R, W, groups_meta)
    return _PROGRAM_CACHE[key]


def _prepare_inputs(external, prev_spikes, membrane, inter_weights,
                    local_kernel, refractory, conn_src, conn_dst):
    Lx, H, W = external.shape
    R = H // NCORES
    conns = [(int(s), int(d)) for s, d in zip(conn_src, conn_dst)]

    groups = _group_kernel_columns(np.asarray(local_kernel, np.float32))
    groups_meta = [(k, d) for k, d, _c in groups]

    # band matrices, hi parts then lo parts, [KR, 2*NG*TH] bf16
    NG = len(groups)
    bands = np.zeros((KR, 2 * NG * TH), ml_dtypes.bfloat16)
    for gi, (_k, _d, col) in enumerate(groups):
        B = _band_matrix(col)
        hi, lo = _split_bf16(B)
        bands[:, gi * TH:(gi + 1) * TH] = hi
        bands[:, (NG + gi) * TH:(NG + gi + 1) * TH] = lo
    iden = np.eye(TH, dtype=ml_dtypes.bfloat16)

    # fp32 threshold plane: out fires iff psum > thr
    ext = np.asarray(external, np.float32)
    mem = np.asarray(membrane, np.float32)
    refr = np.asarray(refractory)
    thr = (BIG * (refr != 0).astype(np.float32)
           - (ext + DECAY * mem)).astype(np.float32)

    # padded bf16 spikes (exact: values {0,1})
    spk = np.zeros((Lx, H + 2 * HALO, W + 2 * HALO), ml_dtypes.bfloat16)
    spk[:, HALO:H + HALO, HALO:W + HALO] = np.asarray(prev_spikes, np.float32)

    w_hi, w_lo = _split_bf16(np.asarray(inter_weights, np.float32))
    wpk = np.concatenate([w_hi, w_lo], axis=2)  # [C, H, 2W]

    in_maps = []
    for c in range(NCORES):
        g0 = c * R
        in_maps.append({
            "spk": np.ascontiguousarray(spk[:, g0:g0 + R + 2 * HALO, :]),
            "thr": np.ascontiguousarray(thr[:, g0:g0 + R, :]),
            "wpk": np.ascontiguousarray(wpk[:, g0:g0 + R, :]),
            "bands": bands,
            "iden": iden,
        })
    return conns, R, W, groups_meta, in_maps


def _ensure_ntff_hook():
    """Inject the missing antenv.axon_hooks module + ctypes NTFF hook so
    trace=True works in this image (profiling only; best-effort)."""
    import types
    try:
        import antenv.axon_hooks  # noqa: F401
        return
    except ImportError:
        pass
    try:
        import antenv
        mod = types.ModuleType("antenv.axon_hooks")
        _h = [None]
        mod.set_axon_ntff_profile_hook = lambda h: _h.__setitem__(0, h)
        mod.get_axon_ntff_profile_hook = lambda: _h[0]
        sys.modules["antenv.axon_hooks"] = mod
        antenv.axon_hooks = mod
        from trn_agent_boot.trn_boot import _ntff_profile_via_ctypes
        hook = _ntff_profile_via_ctypes("/opt/axon/libaxon_pjrt.so")
        if hook is not None:
            _h[0] = hook
    except Exception:
        pass


def kernel(external, prev_spikes, membrane, inter_weights, local_kernel,
           refractory, conn_src, conn_dst, _trace=False):
    if _trace:
        _ensure_ntff_hook()
    conns, R, W, groups_meta, in_maps = _prepare_inputs(
        external, prev_spikes, membrane, inter_weights, local_kernel,
        refractory, conn_src, conn_dst)
    nc = _get_program(conns, R, W, groups_meta)
    res = run_bass_kernel_spmd(nc, in_maps, core_ids=list(range(NCORES)),
                               trace=_trace)
    out = np.concatenate([r["out"].astype(np.float32) for r in res.results],
                         axis=1)
    if _trace:
        kernel._last_results = res
    return out
